# revision 2
# baseline (speedup 1.0000x reference)
"""Causal self-attention (b=4, t=2048, c=1024, 16 heads x 64) on 8 NeuronCores.

Sharding: core j -> batch j//2, head-group g=j%2 (heads 8g..8g+8).
Each core computes qkv for its 8 heads, causal attention, and a partial
output projection (its heads' rows of W_proj). Host sums the two partials
per batch and adds b_proj.

Device layout choices (all matmuls in fp32r at full PE rate):
- x is pre-transposed on host: xT [c=1024, t=2048].
- qT, kT [512, 2048] (head-dim cols on partitions) via lhsT=W, rhs=xT.
- v [2048, 520] (time on partitions) via lhsT=xT, rhs=Wv_aug, where Wv_aug
  has a 65th all-zero column per head whose bias is 1.0 -> constant ones
  column; the AV matmul then yields the softmax denominator in row 64.
- S^T blocks [tk=128, tq=512] = kT_h.T @ qT_h; exp on ScalarE with fused
  1/8 scale; causal zeroing of diagonal blocks via gpsimd affine_select;
  AV accumulates over tk blocks into psum [65, 512].
- Normalization: reciprocal of psum row 64, K=1 fp32 matmul broadcast to
  64 partitions, fused evacuate+multiply into yT.
- Projection per tq chunk: lhsT = yT [128hd, 128tq], rhs = Wp [hd, 1024].
"""

import numpy as np

N_CORES = 8
B, T, C = 4, 2048, 1024
NH, HD = 16, 64  # heads, head dim
HPC = 8  # heads per core
HCOLS = HPC * HD  # 512 head cols per core
VAUG = HPC * (HD + 1)  # 520
TCH = 512  # tq chunk
NCH = T // TCH  # 4 chunks
SCALE = 1.0 / np.sqrt(HD)

_CACHE = {}
LAST_RESULTS = None


def _build_nc():
    from concourse import bacc
    import concourse.mybir as mybir
    import concourse.tile as tile

    f32 = mybir.dt.float32
    f32r = mybir.dt.float32r
    Alu = mybir.AluOpType

    nc = bacc.Bacc("TRN2", target_bir_lowering=False, debug=False, num_devices=N_CORES)

    xT_d = nc.dram_tensor("xT", [C, T], f32r, kind="ExternalInput")
    Wq_d = nc.dram_tensor("Wq", [C, HCOLS], f32r, kind="ExternalInput")
    Wk_d = nc.dram_tensor("Wk", [C, HCOLS], f32r, kind="ExternalInput")
    Wv_d = nc.dram_tensor("Wv", [C, VAUG], f32r, kind="ExternalInput")
    Wp_d = nc.dram_tensor("Wp", [HCOLS, C], f32r, kind="ExternalInput")
    bq_d = nc.dram_tensor("bq", [HCOLS], f32, kind="ExternalInput")
    bk_d = nc.dram_tensor("bk", [HCOLS], f32, kind="ExternalInput")
    bv_d = nc.dram_tensor("bv", [1, VAUG], f32, kind="ExternalInput")
    out_d = nc.dram_tensor("out", [T, C], f32, kind="ExternalOutput")

    KS = C // 128  # 8 contraction subtiles for qkv
    MS = HCOLS // 128  # 4 col subtiles of qT/kT
    NTK = T // 128  # 16 tk blocks

    with tile.TileContext(nc) as tc:
        with (
            tc.tile_pool(name="persist", bufs=1) as persist,
            tc.tile_pool(name="const", bufs=1) as const,
        ):
            # persistent tensors
            qT = persist.tile([128, MS, T], f32r, tag="qT")
            kT = persist.tile([128, MS, T], f32r, tag="kT")
            v = persist.tile([128, NTK, VAUG], f32r, tag="v")
            Wp = persist.tile([128, MS, C], f32r, tag="Wp")
            nc.sync.dma_start(Wp[:], Wp_d.ap().rearrange("(m p) e -> p m e", p=128))

            bq = const.tile([128, MS], f32, tag="bq")
            bk = const.tile([128, MS], f32, tag="bk")
            nc.sync.dma_start(bq[:], bq_d.ap().rearrange("(m p) -> p m", p=128))
            nc.sync.dma_start(bk[:], bk_d.ap().rearrange("(m p) -> p m", p=128))

            ones1 = const.tile([1, 128], f32, tag="ones1")
            nc.vector.memset(ones1[:], 1.0)

            # bv broadcast [1, 520] -> [128, 520] via K=1 fp32 matmuls
            bv_row = const.tile([1, VAUG], f32, tag="bv_row")
            nc.sync.dma_start(bv_row[:], bv_d.ap())
            bvb = const.tile([128, VAUG], f32, tag="bvb")

            with tc.tile_pool(name="pa_psum", bufs=2, space="PSUM") as pap:
                psb = pap.tile([128, 512], f32, tag="pA")
                nc.tensor.matmul(psb[:, :], ones1[0:1, :], bv_row[0:1, 0:512],
                                 start=True, stop=True)
                nc.scalar.copy(bvb[:, 0:512], psb[:, :])
                psb2 = pap.tile([128, 512], f32, tag="pA")
                nc.tensor.matmul(psb2[:, 0:8], ones1[0:1, :], bv_row[0:1, 512:520],
                                 start=True, stop=True)
                nc.scalar.copy(bvb[:, 512:520], psb2[:, 0:8])

                # ---------------- Phase A: qT, kT, v ----------------
                with (
                    tc.tile_pool(name="pa_w", bufs=1) as paw,
                    tc.tile_pool(name="pa_x", bufs=2) as pax,
                ):
                    Wq = paw.tile([128, KS, HCOLS], f32r, tag="Wq")
                    Wk = paw.tile([128, KS, HCOLS], f32r, tag="Wk")
                    Wv = paw.tile([128, KS, VAUG], f32r, tag="Wv")
                    nc.sync.dma_start(Wq[:], Wq_d.ap().rearrange("(ko p) m -> p ko m", p=128))
                    nc.sync.dma_start(Wk[:], Wk_d.ap().rearrange("(ko p) m -> p ko m", p=128))
                    nc.sync.dma_start(Wv[:], Wv_d.ap().rearrange("(ko p) m -> p ko m", p=128))

                    for ch in range(NCH):
                        xt = pax.tile([128, KS, TCH], f32r, tag="xt")
                        nc.sync.dma_start(
                            xt[:],
                            xT_d.ap()[:, ch * TCH:(ch + 1) * TCH]
                            .rearrange("(ko p) t -> p ko t", p=128),
                        )
                        tsl = slice(ch * TCH, (ch + 1) * TCH)
                        # kT chunk, then v chunk, then qT chunk
                        for m in range(MS):
                            ps = pap.tile([128, 512], f32, tag="pA")
                            for k in range(KS):
                                nc.tensor.matmul(ps[:], Wk[:, k, m * 128:(m + 1) * 128],
                                                 xt[:, k, :], start=(k == 0), stop=(k == KS - 1))
                            with nc.allow_low_precision(reason="f32r for PE"):
                                nc.vector.tensor_scalar(
                                    out=kT[:, m, tsl], in0=ps[:],
                                    scalar1=bk[:, m:m + 1], scalar2=None, op0=Alu.add)
                        for ts in range(TCH // 128):
                            tk_i = ch * (TCH // 128) + ts
                            xsl = xt[:, :, ts * 128:(ts + 1) * 128]
                            ps = pap.tile([128, 512], f32, tag="pA")
                            for k in range(KS):
                                nc.tensor.matmul(ps[:], xsl[:, k, :], Wv[:, k, 0:512],
                                                 start=(k == 0), stop=(k == KS - 1))
                            with nc.allow_low_precision(reason="f32r for PE"):
                                nc.vector.tensor_tensor(
                                    v[:, tk_i, 0:512], ps[:], bvb[:, 0:512], Alu.add)
                            ps8 = pap.tile([128, 8], f32, tag="pA8")
                            for k in range(KS):
                                nc.tensor.matmul(ps8[:], xsl[:, k, :], Wv[:, k, 512:520],
                                                 start=(k == 0), stop=(k == KS - 1))
                            with nc.allow_low_precision(reason="f32r for PE"):
                                nc.vector.tensor_tensor(
                                    v[:, tk_i, 512:520], ps8[:], bvb[:, 512:520], Alu.add)
                        for m in range(MS):
                            ps = pap.tile([128, 512], f32, tag="pA")
                            for k in range(KS):
                                nc.tensor.matmul(ps[:], Wq[:, k, m * 128:(m + 1) * 128],
                                                 xt[:, k, :], start=(k == 0), stop=(k == KS - 1))
                            with nc.allow_low_precision(reason="f32r for PE"):
                                nc.vector.tensor_scalar(
                                    out=qT[:, m, tsl], in0=ps[:],
                                    scalar1=bq[:, m:m + 1], scalar2=None, op0=Alu.add)

            # ---------------- Phase B: attention + proj ----------------
            with (
                tc.tile_pool(name="pb", bufs=4) as pb,
                tc.tile_pool(name="pb_y", bufs=2) as pby,
                tc.tile_pool(name="pb_out", bufs=3) as pbo,
                tc.tile_pool(name="ps_s", bufs=3, space="PSUM") as pss,
                tc.tile_pool(name="ps_y", bufs=3, space="PSUM") as psy,
                tc.tile_pool(name="ps_p", bufs=2, space="PSUM") as psp,
            ):
                for ch in range(NCH):
                    tq0 = ch * TCH
                    tsl = slice(tq0, tq0 + TCH)
                    nblk = (ch + 1) * (TCH // 128)
                    yt = pby.tile([128, MS, TCH], f32r, tag="yt")
                    for h in range(HPC):
                        pb_base = (h % 2) * 64
                        hm = h // 2
                        ps_y = psy.tile([128, 512], f32, tag="psY")
                        for i in range(nblk):
                            ps_s = pss.tile([128, 512], f32, tag="psS")
                            nc.tensor.matmul(
                                ps_s[:],
                                kT[pb_base:pb_base + 64, hm, i * 128:(i + 1) * 128],
                                qT[pb_base:pb_base + 64, hm, tsl],
                                start=True, stop=True)
                            es = pb.tile([128, TCH], f32r, tag="es")
                            with nc.allow_low_precision(reason="f32r for PE"):
                                nc.scalar.activation(
                                    es[:], ps_s[:],
                                    mybir.ActivationFunctionType.Exp,
                                    scale=float(SCALE))
                            if i * 128 >= tq0:  # diagonal band block: causal zeroing
                                nc.gpsimd.affine_select(
                                    out=es[:], in_=es[:],
                                    compare_op=Alu.is_ge, fill=0.0,
                                    base=tq0 - i * 128,
                                    pattern=[[1, TCH]], channel_multiplier=-1)
                            nc.tensor.matmul(
                                ps_y[0:65, :], v[:, i, h * 65:(h + 1) * 65], es[:],
                                start=(i == 0), stop=(i == nblk - 1))
                        # normalization
                        rh = pb.tile([1, TCH], f32, tag="rh")
                        nc.vector.reciprocal(rh[0:1, :], ps_y[64:65, :])
                        ps_b = pss.tile([128, 512], f32, tag="psS")
                        nc.tensor.matmul(ps_b[0:64, :], ones1[0:1, 0:64], rh[0:1, :],
                                         start=True, stop=True)
                        bc = pb.tile([64, TCH], f32, tag="bc")
                        nc.scalar.copy(bc[:], ps_b[0:64, :])
                        with nc.allow_low_precision(reason="f32r for PE"):
                            nc.vector.tensor_tensor(
                                yt[pb_base:pb_base + 64, hm, :],
                                ps_y[0:64, :], bc[:], Alu.mult)
                    # projection for this chunk
                    for mt in range(TCH // 128):
                        for n in range(C // 512):
                            ps_o = psp.tile([128, 512], f32, tag="psP")
                            for k in range(MS):
                                nc.tensor.matmul(
                                    ps_o[:],
                                    yt[:, k, mt * 128:(mt + 1) * 128],
                                    Wp[:, k, n * 512:(n + 1) * 512],
                                    start=(k == 0), stop=(k == MS - 1))
                            ot = pbo.tile([128, 512], f32, tag="ot")
                            nc.scalar.copy(ot[:], ps_o[:])
                            nc.sync.dma_start(
                                out_d.ap()[tq0 + mt * 128: tq0 + (mt + 1) * 128,
                                           n * 512:(n + 1) * 512],
                                ot[:])

    nc.compile()
    return nc


def _get_nc():
    if "nc" not in _CACHE:
        _CACHE["nc"] = _build_nc()
    return _CACHE["nc"]


def kernel(x, W_qkv, b_qkv, W_proj, b_proj):
    global LAST_RESULTS
    from concourse.bass_utils import run_bass_kernel_spmd

    x = np.asarray(x, dtype=np.float32)
    W_qkv = np.asarray(W_qkv, dtype=np.float32)
    b_qkv = np.asarray(b_qkv, dtype=np.float32)
    W_proj = np.asarray(W_proj, dtype=np.float32)
    b_proj = np.asarray(b_proj, dtype=np.float32)

    nc = _get_nc()

    in_maps = []
    for j in range(N_CORES):
        bi, g = j // 2, j % 2
        c0 = g * HCOLS
        Wq = np.ascontiguousarray(W_qkv[:, c0:c0 + HCOLS])
        Wk = np.ascontiguousarray(W_qkv[:, C + c0:C + c0 + HCOLS])
        Wv_h = W_qkv[:, 2 * C + c0:2 * C + c0 + HCOLS]  # [C, 512]
        bv_h = b_qkv[2 * C + c0:2 * C + c0 + HCOLS]
        Wv_aug = np.zeros((C, VAUG), dtype=np.float32)
        bv_aug = np.zeros((1, VAUG), dtype=np.float32)
        for h in range(HPC):
            Wv_aug[:, h * 65:h * 65 + 64] = Wv_h[:, h * 64:(h + 1) * 64]
            bv_aug[0, h * 65:h * 65 + 64] = bv_h[h * 64:(h + 1) * 64]
            bv_aug[0, h * 65 + 64] = 1.0
        in_maps.append({
            "xT": np.ascontiguousarray(x[bi].T),
            "Wq": Wq,
            "Wk": Wk,
            "Wv": Wv_aug,
            "Wp": np.ascontiguousarray(W_proj[c0:c0 + HCOLS, :]),
            "bq": np.ascontiguousarray(b_qkv[c0:c0 + HCOLS]),
            "bk": np.ascontiguousarray(b_qkv[C + c0:C + c0 + HCOLS]),
            "bv": bv_aug,
        })

    res = run_bass_kernel_spmd(nc, in_maps, list(range(N_CORES)))
    LAST_RESULTS = res

    out = np.empty((B, T, C), dtype=np.float32)
    for bi in range(B):
        out[bi] = res.results[2 * bi]["out"] + res.results[2 * bi + 1]["out"] + b_proj
    return out


# revision 8
# speedup vs baseline: 1.2074x; 1.2074x over previous
"""Causal self-attention (b=4, t=2048, c=1024, 16 heads x 64) on 8 NeuronCores.

Sharding: core j -> batch j//2, head-group g=j%2 (heads 8g..8g+8).
Each core computes qkv for its 8 heads, causal attention, and a partial
output projection (its heads' rows of W_proj). Host sums the two partials
per batch and adds b_proj.

All matmuls run in fp32r (full PE rate, ~1e-4 rel err). Phases A (qkv
projection) and B (attention + output projection) are interleaved per
512-wide tq chunk: chunk j of attention only needs k/v up to chunk j,
so ScalarE exp work overlaps TensorE projection work of later chunks.

Causal masking: S^T blocks are [tk=128, tq=512]; blocks entirely above
the diagonal are skipped; diagonal-band blocks compute only the valid
tq range (the masked range is never read) plus one [128,128] triangular
mask multiply on VectorE.

Softmax denominator: W_v is augmented with a 65th zero column (bias 1.0)
per head, so the AV matmul's row 64 accumulates sum(exp). Normalization:
ScalarE reciprocal of that row, K=1 fp32 matmul broadcast to 64
partitions, fused in-place multiply on VectorE.
"""

import numpy as np

N_CORES = 8
B, T, C = 4, 2048, 1024
NH, HD = 16, 64
HPC = 8  # heads per core
HCOLS = HPC * HD  # 512
VAUG = HPC * (HD + 1)  # 520
TCH = 512
NCH = T // TCH
SCALE = 1.0 / np.sqrt(HD)

_CACHE = {}
LAST_RESULTS = None


def _build_nc():
    from concourse import bacc
    import concourse.mybir as mybir
    import concourse.tile as tile

    f32 = mybir.dt.float32
    f32r = mybir.dt.float32r
    Alu = mybir.AluOpType
    Act = mybir.ActivationFunctionType

    nc = bacc.Bacc("TRN2", target_bir_lowering=False, debug=False, num_devices=N_CORES)

    xT_d = nc.dram_tensor("xT", [C, T], f32r, kind="ExternalInput")
    Wq_d = nc.dram_tensor("Wq", [C, HCOLS], f32r, kind="ExternalInput")
    Wk_d = nc.dram_tensor("Wk", [C, HCOLS], f32r, kind="ExternalInput")
    Wv_d = nc.dram_tensor("Wv", [C, VAUG], f32r, kind="ExternalInput")
    Wp_d = nc.dram_tensor("Wp", [HCOLS, C], f32r, kind="ExternalInput")
    bq_d = nc.dram_tensor("bq", [HCOLS], f32, kind="ExternalInput")
    bk_d = nc.dram_tensor("bk", [HCOLS], f32, kind="ExternalInput")
    bv_d = nc.dram_tensor("bv", [1, VAUG], f32, kind="ExternalInput")
    tri_d = nc.dram_tensor("TRI", [128, 128], f32r, kind="ExternalInput")
    out_d = nc.dram_tensor("out", [T, C], f32, kind="ExternalOutput")
    import os as _os
    _DBG = _os.environ.get("KDBG", "0") == "1"
    if _DBG:
        kTd = nc.dram_tensor("kTd", [128, HCOLS // 128, T], f32, kind="ExternalOutput")
        vd = nc.dram_tensor("vd", [128, T // 128, VAUG], f32, kind="ExternalOutput")
        qtd = nc.dram_tensor("qtd", [NCH, 128, HCOLS // 128, TCH], f32, kind="ExternalOutput")
        ytd = nc.dram_tensor("ytd", [NCH, 128, HCOLS // 128, TCH], f32, kind="ExternalOutput")

    KS = C // 128  # 8
    MS = HCOLS // 128  # 4
    TSUB = TCH // 128  # 4

    with tile.TileContext(nc) as tc:
        with (
            tc.tile_pool(name="persist", bufs=1) as persist,
            tc.tile_pool(name="stream", bufs=2) as stream,
            tc.tile_pool(name="es_pool", bufs=2) as esp,
            tc.tile_pool(name="small", bufs=2) as small,
            tc.tile_pool(name="pA", bufs=2, space="PSUM") as pA,
            tc.tile_pool(name="pS", bufs=3, space="PSUM") as pS,
            tc.tile_pool(name="pY", bufs=3, space="PSUM") as pY,
        ):
            # small constants first
            bqk = persist.tile([128, 2 * MS], f32, tag="bqk")
            bq = bqk[:, 0:MS]
            bk = bqk[:, MS:2 * MS]
            nc.sync.dma_start(bq, bq_d.ap().rearrange("(m p) -> p m", p=128))
            nc.sync.dma_start(bk, bk_d.ap().rearrange("(m p) -> p m", p=128))
            tri = persist.tile([128, 128], f32r, tag="tri")
            nc.sync.dma_start(tri[:], tri_d.ap())
            # bv_row is transient: borrow an xt streaming slot for it
            bv_row = stream.tile([1, VAUG], f32, tag="xt", name="bv_row")
            nc.sync.dma_start(bv_row[:], bv_d.ap())
            ones1 = persist.tile([1, 128], f32, tag="ones1")
            nc.vector.memset(ones1[:], 1.0)

            # big weights
            Wk = persist.tile([128, KS, HCOLS], f32r, tag="Wk")
            Wv = persist.tile([128, KS, VAUG], f32r, tag="Wv")
            Wq = persist.tile([128, KS, HCOLS], f32r, tag="Wq")
            Wp = persist.tile([128, MS, C], f32r, tag="Wp")
            nc.sync.dma_start(Wk[:], Wk_d.ap().rearrange("(ko p) m -> p ko m", p=128))
            nc.sync.dma_start(Wv[:], Wv_d.ap().rearrange("(ko p) m -> p ko m", p=128))
            nc.sync.dma_start(Wq[:], Wq_d.ap().rearrange("(ko p) m -> p ko m", p=128))
            nc.sync.dma_start(Wp[:], Wp_d.ap().rearrange("(m p) e -> p m e", p=128))

            # persistent activations
            kT = persist.tile([128, MS, T], f32r, tag="kT")
            v = persist.tile([128, T // 128, VAUG], f32r, tag="v")

            # bv broadcast [1, VAUG] -> [128, VAUG]
            bvb = persist.tile([128, VAUG], f32, tag="bvb")
            ps = pA.tile([128, 512], f32, tag="pA")
            nc.tensor.matmul(ps[:, :], ones1[0:1, :], bv_row[0:1, 0:512],
                             start=True, stop=True)
            nc.scalar.copy(bvb[:, 0:512], ps[:, :])
            ps = pA.tile([128, 512], f32, tag="pA")
            nc.tensor.matmul(ps[:, 0:8], ones1[0:1, :], bv_row[0:1, 512:520],
                             start=True, stop=True)
            nc.scalar.copy(bvb[:, 512:520], ps[:, 0:8])

            def phase_a_groups(ch):
                """Return list of emit-callables for qkv production of chunk ch."""
                tsl = slice(ch * TCH, (ch + 1) * TCH)
                xt = stream.tile([128, KS, TCH], f32r, tag="xt")
                nc.sync.dma_start(
                    xt[:],
                    xT_d.ap()[:, ch * TCH:(ch + 1) * TCH]
                    .rearrange("(ko p) t -> p ko t", p=128),
                )
                qt = stream.tile([128, MS, TCH], f32r, tag="qt")
                groups = []

                def k_group(m):
                    def emit():
                        ps = pA.tile([128, 512], f32, tag="pA")
                        for k in range(KS):
                            nc.tensor.matmul(ps[:], Wk[:, k, m * 128:(m + 1) * 128],
                                             xt[:, k, :], start=(k == 0), stop=(k == KS - 1))
                        with nc.allow_low_precision(reason="f32r"):
                            nc.vector.tensor_scalar(
                                out=kT[:, m, tsl], in0=ps[:],
                                scalar1=bk[:, m:m + 1], scalar2=None, op0=Alu.add)
                    return emit

                def v_group(ts):
                    def emit():
                        tk_i = ch * TSUB + ts
                        xsl = xt[:, :, ts * 128:(ts + 1) * 128]
                        ps = pA.tile([128, 512], f32, tag="pA")
                        for k in range(KS):
                            nc.tensor.matmul(ps[:], xsl[:, k, :], Wv[:, k, 0:512],
                                             start=(k == 0), stop=(k == KS - 1))
                        with nc.allow_low_precision(reason="f32r"):
                            nc.vector.tensor_tensor(
                                v[:, tk_i, 0:512], ps[:], bvb[:, 0:512], Alu.add)
                        ps8 = pA.tile([128, 512], f32, tag="pA")
                        for k in range(KS):
                            nc.tensor.matmul(ps8[:, 0:8], xsl[:, k, :], Wv[:, k, 512:520],
                                             start=(k == 0), stop=(k == KS - 1))
                        with nc.allow_low_precision(reason="f32r"):
                            nc.vector.tensor_tensor(
                                v[:, tk_i, 512:520], ps8[:, 0:8], bvb[:, 512:520], Alu.add)
                    return emit

                def q_group(m):
                    def emit():
                        ps = pA.tile([128, 512], f32, tag="pA")
                        for k in range(KS):
                            nc.tensor.matmul(ps[:], Wq[:, k, m * 128:(m + 1) * 128],
                                             xt[:, k, :], start=(k == 0), stop=(k == KS - 1))
                        with nc.allow_low_precision(reason="f32r"):
                            nc.vector.tensor_scalar(
                                out=qt[:, m, :], in0=ps[:],
                                scalar1=bq[:, m:m + 1], scalar2=None, op0=Alu.add)
                    return emit

                for m in range(MS):
                    groups.append(k_group(m))
                for ts in range(TSUB):
                    groups.append(v_group(ts))
                for m in range(MS):
                    groups.append(q_group(m))
                return qt, groups

            def emit_pair(ch, qt, p):
                """Attention for head pair (2p, 2p+1) of chunk ch."""
                nblk = (ch + 1) * TSUB
                hA, hB = 2 * p, 2 * p + 1
                psy = {}
                for h in (hA, hB):
                    psy[h] = pY.tile([128, 512], f32, tag="pY", name=f"psy{h}")
                es = {}
                for i in range(nblk):
                    diag_k = i - ch * TSUB  # >=0 in diagonal band
                    vs = 128 * diag_k if diag_k > 0 else 0
                    n = TCH - vs
                    pss = {}
                    for h in (hA, hB):
                        pb = (h % 2) * 64
                        hm = h // 2
                        pss[h] = pS.tile([128, 512], f32, tag="pS", name=f"pss{h}")
                        nc.tensor.matmul(
                            pss[h][:, vs:TCH],
                            kT[pb:pb + 64, hm, i * 128:(i + 1) * 128],
                            qt[pb:pb + 64, hm, vs:TCH],
                            start=True, stop=True)
                    for h in (hA, hB):
                        es[h] = esp.tile([128, TCH], f32r, tag="es", name=f"es{h}")
                        with nc.allow_low_precision(reason="f32r"):
                            nc.scalar.activation(
                                es[h][:, vs:TCH], pss[h][:, vs:TCH],
                                Act.Exp, scale=float(SCALE))
                        if diag_k >= 0:
                            with nc.allow_low_precision(reason="f32r"):
                                nc.vector.tensor_tensor(
                                    es[h][:, vs:vs + 128], es[h][:, vs:vs + 128],
                                    tri[:], Alu.mult)
                    for h in (hA, hB):
                        nc.tensor.matmul(
                            psy[h][0:65, vs:TCH],
                            v[:, i, h * 65:(h + 1) * 65],
                            es[h][:, vs:TCH],
                            start=(i == 0), stop=(i == nblk - 1))
                # normalization
                for h in (hA, hB):
                    pb = (h % 2) * 64
                    hm = h // 2
                    # shifted DVE copy of the l row to sbuf base 0 (the custom
                    # reciprocal op misreads psum at partition base 64)
                    lrow = small.tile([1, TCH], f32, tag="ot", name=f"lrow{h}")
                    nc.vector.tensor_copy(lrow[0:1, :], psy[h][64:65, :])
                    rrow = small.tile([1, TCH], f32, tag="rrow")
                    nc.vector.reciprocal_approx_fast(rrow[0:1, :], lrow[0:1, :])
                    psb = pS.tile([128, 512], f32, tag="pS")
                    nc.tensor.matmul(psb[0:64, :], ones1[0:1, 0:64], rrow[0:1, :],
                                     start=True, stop=True)
                    with nc.allow_low_precision(reason="f32r"):
                        nc.vector.tensor_copy(yt[pb:pb + 64, hm, :], psy[h][0:64, :])
                        nc.vector.tensor_tensor(
                            yt[pb:pb + 64, hm, :], yt[pb:pb + 64, hm, :],
                            psb[0:64, :], Alu.mult)

            def emit_proj(ch):
                tq0 = ch * TCH
                for mt in range(TSUB):
                    for n in range(C // 512):
                        ps = pA.tile([128, 512], f32, tag="pA")
                        for k in range(MS):
                            nc.tensor.matmul(
                                ps[:],
                                yt[:, k, mt * 128:(mt + 1) * 128],
                                Wp[:, k, n * 512:(n + 1) * 512],
                                start=(k == 0), stop=(k == MS - 1))
                        ot = small.tile([128, 512], f32, tag="ot")
                        nc.vector.tensor_copy(ot[:], ps[:])
                        nc.sync.dma_start(
                            out_d.ap()[tq0 + mt * 128: tq0 + (mt + 1) * 128,
                                       n * 512:(n + 1) * 512],
                            ot[:])

            # ---- main schedule: interleave A(ch+1) into B(ch) ----
            qt_cur, groups = phase_a_groups(0)
            for g in groups:
                g()
            for ch in range(NCH):
                yt = stream.tile([128, MS, TCH], f32r, tag="yt")
                if ch + 1 < NCH:
                    qt_next, next_groups = phase_a_groups(ch + 1)
                else:
                    qt_next, next_groups = None, []
                gi = 0
                for p in range(4):
                    emit_pair(ch, qt_cur, p)
                    # interleave phase-A work of the next chunk
                    take = (len(next_groups) * (p + 1) + 3) // 4 - gi
                    for _ in range(take):
                        next_groups[gi]()
                        gi += 1
                if _DBG:
                    nc.sync.dma_start(qtd.ap()[ch], qt_cur[:].bitcast(f32))
                    nc.sync.dma_start(ytd.ap()[ch], yt[:].bitcast(f32))
                emit_proj(ch)
                qt_cur = qt_next

            if _DBG:
                nc.sync.dma_start(kTd.ap(), kT[:].bitcast(f32))
                nc.sync.dma_start(vd.ap(), v[:].bitcast(f32))

    nc.compile()
    return nc


def _get_nc():
    if "nc" not in _CACHE:
        _CACHE["nc"] = _build_nc()
    return _CACHE["nc"]


def kernel(x, W_qkv, b_qkv, W_proj, b_proj):
    global LAST_RESULTS
    from concourse.bass_utils import run_bass_kernel_spmd

    x = np.asarray(x, dtype=np.float32)
    W_qkv = np.asarray(W_qkv, dtype=np.float32)
    b_qkv = np.asarray(b_qkv, dtype=np.float32)
    W_proj = np.asarray(W_proj, dtype=np.float32)
    b_proj = np.asarray(b_proj, dtype=np.float32)

    nc = _get_nc()

    tri = np.tril(np.ones((128, 128), dtype=np.float32)).T.copy()  # tri[p,f]=1 iff p<=f

    in_maps = []
    for j in range(N_CORES):
        bi, g = j // 2, j % 2
        c0 = g * HCOLS
        Wv_h = W_qkv[:, 2 * C + c0:2 * C + c0 + HCOLS]
        bv_h = b_qkv[2 * C + c0:2 * C + c0 + HCOLS]
        Wv_aug = np.zeros((C, VAUG), dtype=np.float32)
        bv_aug = np.zeros((1, VAUG), dtype=np.float32)
        for h in range(HPC):
            Wv_aug[:, h * 65:h * 65 + 64] = Wv_h[:, h * 64:(h + 1) * 64]
            bv_aug[0, h * 65:h * 65 + 64] = bv_h[h * 64:(h + 1) * 64]
            bv_aug[0, h * 65 + 64] = 1.0
        in_maps.append({
            "xT": np.ascontiguousarray(x[bi].T),
            "Wq": np.ascontiguousarray(W_qkv[:, c0:c0 + HCOLS]),
            "Wk": np.ascontiguousarray(W_qkv[:, C + c0:C + c0 + HCOLS]),
            "Wv": Wv_aug,
            "Wp": np.ascontiguousarray(W_proj[c0:c0 + HCOLS, :]),
            "bq": np.ascontiguousarray(b_qkv[c0:c0 + HCOLS]),
            "bk": np.ascontiguousarray(b_qkv[C + c0:C + c0 + HCOLS]),
            "bv": bv_aug,
            "TRI": tri,
        })

    res = run_bass_kernel_spmd(nc, in_maps, list(range(N_CORES)))
    LAST_RESULTS = res

    out = np.empty((B, T, C), dtype=np.float32)
    for bi in range(B):
        out[bi] = res.results[2 * bi]["out"] + res.results[2 * bi + 1]["out"] + b_proj
    return out
